# revision 52
# baseline (speedup 1.0000x reference)
"""DebertaV2 disentangled attention block on 8 TRN2 NeuronCores (Bass/Tile).

Head-sharded tensor parallel: 2 heads per core. Host does layout-only prep
(transpose / bucket-reversal / dtype cast); all FLOPs run on device.
ReduceScatter after the output dense; per-core LayerNorm on its 128 rows.

Perf notes (cost-model driven):
- All HBM loads are single batched DMAs with multi-dim access patterns
  (per-dma fixed cost ~1.2us serialized on the issuing queue + HWDGE).
- Skew gather (c2p/p2c band -> per-row shifted read) is batched 4 tiles
  per DMA through a DRAM scratch with row stride W_WIN-1.
- Skew block matmuls write one 3-bank PSUM tile -> single PSUM->SBUF copy.
- Softmax reciprocal is broadcast across partitions with a K=1 matmul
  instead of a DRAM roundtrip.
- DMA issue spread across SP (sync), ACT (scalar) and Pool (gpsimd).
"""

import math

import numpy as np

H = 16
D = 64
HID = 1024
N = 1024
K = 1024
EPS = 1e-7
NCORES = 8
HPC = H // NCORES  # heads per core = 2
DPC = HPC * D      # head dims per core = 128
SCALE = 1.0 / math.sqrt(3.0 * D)  # applied inside exp()

W_WIN = 1151       # skew window width (127 + 1024)
SCR_STRIDE = 128 * W_WIN
P = 128

_CACHE = {}


def _build():
    import concourse.bass as bass
    import concourse.mybir as mybir
    import concourse.tile as tile
    from concourse import bacc
    from concourse.masks import make_identity
    from contextlib import ExitStack

    f32 = mybir.dt.float32
    bf16 = mybir.dt.bfloat16
    fp8 = mybir.dt.float8e4

    nc = bacc.Bacc(None, target_bir_lowering=False, debug=False)
    names = {}

    with tile.TileContext(nc) as tc, ExitStack() as es:
        dio = es.enter_context(tc.tile_pool(name="dram_io", bufs=1, space="DRAM"))
        dwork = es.enter_context(tc.tile_pool(name="dram_work", bufs=1, space="DRAM"))

        def din(nm, shape, dt=bf16):
            t = dio.tile(shape, dt, kind="ExternalInput", name=nm, tag=nm)
            names[nm] = t.name
            return t

        hsT = din("hsT", (HID, N))            # hs[0].T, bf16
        relTn = din("relTn", (HID, 2 * K), fp8)  # rel.T, fp8 (pos_q; pos_k
        #                                       streams it column-reversed)
        wqT = din("wqT", (HID, DPC))
        wkT = din("wkT", (HID, DPC))
        wvT = din("wvT", (HID, DPC))
        wpkT = din("wpkT", (HID, DPC), fp8)   # prescaled x16 on host
        wpqT = din("wpqT", (HID, DPC), fp8)   # prescaled x16 on host
        woT = din("woT", (HID, HID))          # full Wo^T (dense runs
        #                                       post-AllToAll on own rows)
        hs_rows = din("hs_rows", (P, HID), f32)
        b5 = din("b5", (5, DPC), f32)         # bq|bk|bv|bpk|bpq (per-core slice)
        b3 = din("b3", (3, HID), f32)         # bo|ln_g|ln_b

        out_t = dio.tile((P, HID), bf16, kind="ExternalOutput", name="out", tag="out")
        names["out"] = out_t.name

        # AllToAll buffers: shard j = my ctx block [128 dpc, 128 i] for core j
        ctx_send = dwork.tile((NCORES * P * P,), bf16, name="ctx_send",
                              tag="ctx_send")
        ctx_rcv = dwork.tile((NCORES * P * P,), bf16, name="ctx_rcv",
                             tag="ctx_rcv")

        # ---- SBUF pools -------------------------------------------------
        wt = es.enter_context(tc.tile_pool(name="wt", bufs=1))
        work = es.enter_context(tc.tile_pool(name="work", bufs=1))
        psS = es.enter_context(tc.tile_pool(name="psS", bufs=2, space="PSUM"))
        psB = es.enter_context(tc.tile_pool(name="psB", bufs=1, space="PSUM"))
        psK = es.enter_context(tc.tile_pool(name="psK", bufs=4, space="PSUM"))

        Iden = mybir.ActivationFunctionType.Identity
        Exp = mybir.ActivationFunctionType.Exp
        Sqrt = mybir.ActivationFunctionType.Sqrt
        ADD = mybir.AluOpType.add
        MUL = mybir.AluOpType.mult
        SUB = mybir.AluOpType.subtract

        # ---- upfront batched loads --------------------------------------
        # Spread across SP/ACT/Pool queues so the streams run concurrently;
        # the pos-projection inputs (rel + wpk/wpq) gate the longest chain.
        w_all = {}

        def load_w(kk, src, eng, dt=bf16):
            t = wt.tile([P, 8 * DPC], dt, name=f"w{kk}", tag=f"w{kk}")
            eng.dma_start(
                t[:], bass.AP(src[:].tensor, src[:].offset,
                              [[DPC, P], [P * DPC, 8], [1, DPC]]))
            w_all[kk] = t

        load_w("pk", wpkT, nc.sync, fp8)
        load_w("pq", wpqT, nc.sync, fp8)

        # hidden states at the bottom of the stack (freed last)
        hsT_all, hsT_free = tc.tile([P, 8 * N], bf16, name="hsT_all")
        rn0, rn0_f = tc.tile([P, 4 * 2 * K], fp8, name="relTn0")
        rn1, rn1_f = tc.tile([P, 4 * 2 * K], fp8, name="relTn1")
        nc.gpsimd.dma_start(
            hsT_all[:], bass.AP(hsT[:].tensor, hsT[:].offset,
                                [[N, P], [P * N, 8], [1, N]]))

        # rel embedding halves as [128 hid-k, 4 tiles x 2048 pos]
        for half, t, eng in ((0, rn0, nc.sync), (1, rn1, nc.scalar)):
            base = relTn[:].offset + half * 4 * P * 2 * K
            eng.dma_start(
                t[:], bass.AP(relTn[:].tensor, base,
                              [[2 * K, P], [P * 2 * K, 4], [1, 2 * K]]))

        load_w("q", wqT, nc.sync)
        load_w("k", wkT, nc.scalar)
        load_w("v", wvT, nc.gpsimd)

        b5_sb = wt.tile([P, 5], f32, name="b5_sb", tag="b5_sb")
        nc.sync.dma_start(
            b5_sb[:], bass.AP(b5[:].tensor, b5[:].offset, [[1, P], [P, 5]]))

        ident = wt.tile([P, P], bf16, name="ident", tag="ident")
        make_identity(nc, ident[:])
        id8 = wt.tile([P, P], fp8, name="id8", tag="id8")
        nc.scalar.copy(id8[:], ident[:])
        ones1 = wt.tile([1, 64], f32, name="ones1", tag="ones1")
        nc.vector.memset(ones1[:], 1.0)

        # late loads (only needed at the output stage)
        woT_sb = wt.tile([P, 8 * HID], bf16, name="woT_sb", tag="woT_sb")
        nc.scalar.dma_start(
            woT_sb[:], bass.AP(woT[:].tensor, woT[:].offset,
                               [[HID, P], [P * HID, 8], [1, HID]]))
        hsr_sb = wt.tile([P, HID], f32, name="hsr_sb", tag="hsr_sb")
        nc.gpsimd.dma_start(hsr_sb[:], hs_rows[:])
        bc_all = wt.tile([P, 3 * HID], f32, name="bc_all", tag="bc_all")
        nc.gpsimd.dma_start(
            bc_all[:], bass.AP(b3[:].tensor, b3[:].offset, [[0, P], [1, 3 * HID]]))
        # hs residual + bo, precomputed off the post-collective tail
        hsb = wt.tile([P, HID], f32, name="hsb", tag="hsb")
        nc.vector.tensor_add(hsb[:], hsr_sb[:], bc_all[:, 0:HID])

        # ---- projections -------------------------------------------------
        qT = wt.tile([P, N], bf16, name="qT", tag="qT")
        kT = wt.tile([P, N], bf16, name="kT", tag="kT")
        pkT = wt.tile([P, 2 * K], bf16, name="pkT", tag="pkT")
        pqT = wt.tile([P, 2 * K], bf16, name="pqT", tag="pqT")

        DblRow = mybir.MatmulPerfMode.DoubleRow
        PDESC = 1.0 / 16.0  # undo the x16 host prescale of fp8 pos weights
        pdesc_t = wt.tile([P, 1], f32, name="pdesc_t", tag="pdesc_t")
        nc.vector.memset(pdesc_t[:], PDESC)

        def bias_bcast(col, width=512):
            # b5_sb column broadcast along the free dim (0-stride read)
            full = b5_sb[:]
            return bass.AP(full.tensor, full.offset + col, [[5, P], [0, width]])

        def pos_post(dst, ps, col):
            # (ps * 1/16) + bias on DVE: the front is ACT-bound, DVE is idle
            nc.vector.scalar_tensor_tensor(dst, ps[:], pdesc_t[:],
                                           bias_bcast(col), op0=MUL, op1=ADD)

        # pos_k projection first (it gates the skew chains): stream relTn
        # column-REVERSED so pkT comes out in the bucket-reversed layout the
        # c2p skew gather needs. fp8 + DoubleRow, 2x PE throughput.
        rel_w = 4 * 2 * K  # free width of a rel half tile

        def rel_rev_rhs(rn, t4, c0):
            full = rn[:]
            off = full.offset + 2 * K * t4 + (2 * K - 1) - c0
            return bass.AP(full.tensor, off, [[rel_w, P], [-1, 512]])

        def w_pair(w, j):
            # [K, 2 k-tiles, M] stationary pair for DoubleRow
            full = w[:]
            return bass.AP(full.tensor, full.offset + 2 * DPC * j,
                           [[8 * DPC, P], [DPC, 2], [1, DPC]])

        def rel_pair(j, c0, rev):
            # [K, 2 k-tiles, 512] moving pair; k-tile pairs never straddle
            # the two rel halves
            rn = rn0 if j < 2 else rn1
            full = rn[:]
            t4 = 2 * (j % 2)
            if rev:
                off = full.offset + 2 * K * t4 + (2 * K - 1) - c0
                return bass.AP(full.tensor, off,
                               [[rel_w, P], [2 * K, 2], [-1, 512]])
            off = full.offset + 2 * K * t4 + c0
            return bass.AP(full.tensor, off,
                           [[rel_w, P], [2 * K, 2], [1, 512]])

        for c0 in range(0, 2 * K, 512):
            ps = psS.tile([P, 512], f32, name="ppk", tag="ps512", bufs=2)
            for j in range(4):
                nc.tensor.matmul(ps[:], w_pair(w_all["pk"], j),
                                 rel_pair(j, c0, True),
                                 start=(j == 0), stop=(j == 3),
                                 perf_mode=DblRow)
            # reversed stream: output col j of this chunk is pos 2047-c0-j,
            # i.e. pkT[:, c] = pos_k[2047-c]
            pos_post(pkT[:, c0:c0 + 512], ps, 3)

        def project(dst, w, src_all, tilew, bcol_idx):
            for c0 in range(0, tilew, 512):
                ps = psS.tile([P, 512], f32, name="pp", tag="ps512", bufs=2)
                for t in range(8):
                    nc.tensor.matmul(ps[:], w[:, DPC * t:DPC * (t + 1)],
                                     src_all[:, tilew * t + c0:tilew * t + c0 + 512],
                                     start=(t == 0), stop=(t == 7))
                # bias add on DVE (broadcast column), keeping ACT free
                nc.vector.tensor_add(dst[:, c0:c0 + 512], ps[:],
                                     bias_bcast(bcol_idx))

        # qT right after pkT: together they unblock the first skew chain
        project(qT, w_all["q"], hsT_all, N, 0)

        for c0 in range(0, 2 * K, 512):
            ps = psS.tile([P, 512], f32, name="ppq", tag="ps512", bufs=2)
            for j in range(4):
                nc.tensor.matmul(ps[:], w_pair(w_all["pq"], j),
                                 rel_pair(j, c0, False),
                                 start=(j == 0), stop=(j == 3),
                                 perf_mode=DblRow)
            pos_post(pqT[:, c0:c0 + 512], ps, 4)
        rn1_f()
        rn0_f()

        project(kT, w_all["k"], hsT_all, N, 1)

        # v in [j, d] layout + ones column per head: va[jt] is [128, 132]
        va = []
        for jt in range(8):
            t = wt.tile([P, 132], bf16, name=f"va{jt}", tag=f"va{jt}")
            ps = psS.tile([P, DPC], f32, name="pv", tag="ps512", bufs=2)
            for kt in range(8):
                nc.tensor.matmul(ps[:], hsT_all[:, N * kt + P * jt:N * kt + P * (jt + 1)],
                                 w_all["v"][:, DPC * kt:DPC * (kt + 1)],
                                 start=(kt == 0), stop=(kt == 7))
            nc.vector.tensor_copy(t[:, 0:64], ps[:, 0:64])
            nc.vector.tensor_copy(t[:, 66:130], ps[:, 64:128])
            nc.vector.memset(t[:, 64:65], 1.0)
            nc.vector.memset(t[:, 130:131], 1.0)
            va.append(t)
        hsT_free()

        # ---- attention per head -----------------------------------------
        ctxT = wt.tile([P, N], bf16, name="ctxT", tag="ctxT")

        copy_flip = [0, 0]

        def skew_batch(lhs, src_T, hd, tagp, g):
            """g[p, r*1024 + x] = lhs[hd][:, 128r+p] . src_T[hd][:, w0_r + 127-p+x]

            Band matmuls in 1-bank PSUM chunks (rotating 4-slot pool, fp8
            SBUF staging, copies split across DVE/ACT) -> 4-block batched
            DRAM roundtrip with row stride W_WIN-1 (the per-partition
            diagonal shift)."""
            for half in (0, 1):
                blk, blk_f = tc.tile([P, 4 * W_WIN], fp8, name=f"blk_{tagp}{half}")
                for ri in range(4):
                    r = half * 4 + ri
                    w0 = (896 if tagp == "c" else 897) - 128 * r
                    for (c0, w) in ((0, 512), (512, 512), (1024, 127)):
                        ps = psK.tile([P, 512], f32, name="bps", tag="bps",
                                      bufs=4)
                        nc.tensor.matmul(
                            ps[:, 0:w],
                            lhs[hd, 128 * r:128 * (r + 1)],
                            src_T[hd, w0 + c0:w0 + c0 + w],
                            start=True, stop=True)
                        dst = blk[:, W_WIN * ri + c0:W_WIN * ri + c0 + w]
                        # balance PSUM->SBUF copies: A->DVE, B mostly ACT
                        # (every 4th to DVE), C splits evenly
                        if c0 == 0:
                            use_dve = True
                        elif c0 == 512:
                            use_dve = copy_flip[1] % 4 == 3
                            copy_flip[1] += 1
                        else:
                            use_dve = copy_flip[0] % 2 == 0
                            copy_flip[0] += 1
                        if use_dve:
                            nc.vector.tensor_copy(dst, ps[:, 0:w])
                        else:
                            nc.scalar.copy(dst, ps[:, 0:w])
                scr = dwork.tile((4 * P * W_WIN,), fp8, name=f"scr_{tagp}{half}",
                                 tag="scr", bufs=4)
                hdl = scr[:].tensor
                base = scr[:].offset
                nc.sync.dma_start(
                    bass.AP(hdl, base, [[W_WIN, P], [SCR_STRIDE, 4], [1, W_WIN]]),
                    blk[:])
                blk_f()
                nc.sync.dma_start(
                    g[:, half * 4096:(half + 1) * 4096],
                    bass.AP(hdl, base + 127,
                            [[W_WIN - 1, P], [SCR_STRIDE, 4], [1, N]]))

        gs = []
        for h in range(HPC):
            hd = slice(64 * h, 64 * h + 64)
            # c2p gathered: i-tile r at cols [r*1024, (r+1)*1024), [i, j] layout
            gc, gc_f = tc.tile([P, 8 * N], fp8, name=f"g_c{h}")
            skew_batch(qT, pkT, hd, "c", gc)
            # p2cT gathered: j-tile jt at cols [jt*1024, ...), [j, i] layout
            gp, gp_f = tc.tile([P, 8 * N], fp8, name=f"g_p{h}")
            skew_batch(kT, pqT, hd, "p", gp)
            gs.append((gc, gc_f, gp, gp_f))

        for h in range(HPC):
            hd = slice(64 * h, 64 * h + 64)
            gc, _, gp, _ = gs[h]

            pb = psB.tile([65, N], f32, name="pb", tag="pb", bufs=1)
            for jt in range(8):
                e = work.tile([P, N], bf16, name=f"expST{jt}", tag="expST",
                              bufs=2)
                for c in range(2):
                    st = psS.tile([P, 512], f32, name="st", tag="ps512", bufs=2)
                    nc.tensor.matmul(st[:], kT[hd, 128 * jt:128 * (jt + 1)],
                                     qT[hd, 512 * c:512 * (c + 1)],
                                     start=True, stop=False)
                    for rr in range(4):
                        r = 4 * c + rr
                        nc.tensor.matmul(
                            st[:, 128 * rr:128 * (rr + 1)],
                            gc[:, N * r + 128 * jt:N * r + 128 * (jt + 1)],
                            id8[:], start=False, stop=False)
                    # p2cT folded into the PSUM accumulation (id8.T @ gp = gp)
                    nc.tensor.matmul(
                        st[:], id8[:],
                        gp[:, N * jt + 512 * c:N * jt + 512 * (c + 1)],
                        start=False, stop=True)
                    nc.scalar.activation(e[:, 512 * c:512 * (c + 1)], st[:],
                                         Exp, scale=SCALE)
                for c in range(2):
                    nc.tensor.matmul(pb[:, 512 * c:512 * (c + 1)],
                                     va[jt][:, 66 * h:66 * h + 65],
                                     e[:, 512 * c:512 * (c + 1)],
                                     start=(jt == 0), stop=(jt == 7))

            # softmax normalize; reciprocal broadcast across partitions via
            # a K=1 matmul (outer product with a ones column)
            recip = work.tile([1, N], f32, name="recip", tag="recip", bufs=2)
            nc.vector.reciprocal(recip[:], pb[64:65, :])
            ctmp = work.tile([64, N], bf16, name="ctmp", tag="ctmp", bufs=2)
            rbs = work.tile([64, N], f32, name="rbs", tag="rbs", bufs=2)
            for c in range(2):
                rb = psS.tile([64, 512], f32, name="rb", tag="ps512", bufs=2)
                nc.tensor.matmul(rb[:], ones1[:], recip[:, 512 * c:512 * (c + 1)],
                                 start=True, stop=True)
                # DVE can read only one PSUM operand; stage rb in SBUF
                nc.scalar.copy(rbs[:, 512 * c:512 * (c + 1)], rb[:])
                nc.vector.tensor_mul(ctmp[:, 512 * c:512 * (c + 1)],
                                     pb[0:64, 512 * c:512 * (c + 1)],
                                     rbs[:, 512 * c:512 * (c + 1)])
            nc.scalar.activation(ctxT[hd, :], ctmp[:], Iden, bias=b5_sb[hd, 2:3])

            # stage this head's half of every A2A shard now — head 0's send
            # overlaps head 1's score loop
            csh = ctx_send[:].tensor
            csb = ctx_send[:].offset
            nc.sync.dma_start(
                bass.AP(csh, csb + 64 * h * P,
                        [[P, 64], [P * P, NCORES], [1, P]]),
                ctxT[hd, :])

        for gc, gc_f, gp, gp_f in reversed(gs):
            gp_f()
            gc_f()

        # ---- AllToAll of per-head-normalized context ---------------------
        # shard j (contiguous 32KB) = my [128 dpc, 128 i] block for core j;
        # after A2A, block j' = core j's dpc dims for MY 128 rows.
        nc.gpsimd.collective_compute(
            "AllToAll", mybir.AluOpType.bypass,
            replica_groups=[list(range(NCORES))],
            ins=[ctx_send[:]], outs=[ctx_rcv[:]])
        ctx_sb = wt.tile([P, 8 * P], bf16, name="ctx_sb", tag="ctx_sb")
        crh = ctx_rcv[:].tensor
        crb = ctx_rcv[:].offset
        for rh in (0, 1):
            nc.sync.dma_start(
                ctx_sb[:, rh * 4 * P:(rh + 1) * 4 * P],
                bass.AP(crh, crb + rh * 4 * P * P,
                        [[P, P], [P * P, 4], [1, P]]))

        # ---- output dense on own 128 rows + residual ---------------------
        x = wt.tile([P, HID], f32, name="x", tag="x")
        for c in range(2):
            po = psS.tile([P, 512], f32, name="po", tag="ps512", bufs=2)
            for j in range(8):
                nc.tensor.matmul(po[:], ctx_sb[:, P * j:P * (j + 1)],
                                 woT_sb[:, HID * j + 512 * c:
                                        HID * j + 512 * (c + 1)],
                                 start=(j == 0), stop=(j == 7))
            # residual + bo folded in from the precomputed hsb tile
            nc.vector.tensor_add(x[:, 512 * c:512 * (c + 1)], po[:],
                                 hsb[:, 512 * c:512 * (c + 1)])

        stats = wt.tile([P, 2, 6], f32, name="stats", tag="stats")
        mv = wt.tile([P, 2], f32, name="mv", tag="mv")
        for s in range(2):
            nc.vector.bn_stats(stats[:, s, :], x[:, 512 * s:512 * (s + 1)])
        nc.vector.bn_aggr(mv[:], stats[:])
        epsb = wt.tile([P, 1], f32, name="epsb", tag="epsb")
        nc.vector.memset(epsb[:], EPS)
        std = wt.tile([P, 1], f32, name="std", tag="std")
        nc.scalar.activation(std[:], mv[:, 1:2], Sqrt, bias=epsb[:])
        rstd = wt.tile([P, 1], f32, name="rstd", tag="rstd")
        nc.vector.reciprocal(rstd[:], std[:])

        t1 = wt.tile([P, HID], f32, name="t1", tag="t1")
        nc.vector.scalar_tensor_tensor(t1[:], x[:], mv[:, 0:1],
                                       bc_all[:, HID:2 * HID],
                                       op0=SUB, op1=MUL)
        yout = wt.tile([P, HID], bf16, name="yout", tag="yout")
        nc.vector.scalar_tensor_tensor(yout[:], t1[:], rstd[:],
                                       bc_all[:, 2 * HID:3 * HID],
                                       op0=MUL, op1=ADD)
        nc.sync.dma_start(out_t[:], yout[:])

    nc.compile()
    return nc, names


def _get_compiled():
    if "nc" not in _CACHE:
        nc, names = _build()
        _CACHE["nc"] = nc
        _CACHE["names"] = names
    return _CACHE["nc"], _CACHE["names"]


def _prep_in_maps(inputs):
    import ml_dtypes

    bf = ml_dtypes.bfloat16
    f8 = ml_dtypes.float8_e4m3
    hs = np.asarray(inputs["hidden_states"], np.float32)[0]      # (N, HID)
    rel = np.asarray(inputs["rel_embeddings"], np.float32)       # (2K, HID)
    hsT = np.ascontiguousarray(hs.T).astype(bf)
    relTn = np.ascontiguousarray(rel.T).astype(f8)
    b3 = np.ascontiguousarray(np.stack([
        np.asarray(inputs["bo"], np.float32),
        np.asarray(inputs["ln_g"], np.float32),
        np.asarray(inputs["ln_b"], np.float32),
    ]))
    woTf = np.ascontiguousarray(np.asarray(inputs["Wo"], np.float32).T).astype(bf)

    def wT(w, r, dt=bf, scale=1.0):
        w = np.asarray(w, np.float32) * scale
        return np.ascontiguousarray(w[DPC * r:DPC * (r + 1), :].T).astype(dt)

    in_maps = []
    for r in range(NCORES):
        b5 = np.ascontiguousarray(np.stack([
            np.asarray(inputs[k], np.float32)[DPC * r:DPC * (r + 1)]
            for k in ("bq", "bk", "bv", "bpk", "bpq")
        ]))
        m = {
            "hsT": hsT,
            "relTn": relTn,
            "wqT": wT(inputs["Wq"], r),
            "wkT": wT(inputs["Wk"], r),
            "wvT": wT(inputs["Wv"], r),
            "wpkT": wT(inputs["Wpk"], r, f8, 16.0),
            "wpqT": wT(inputs["Wpq"], r, f8, 16.0),
            "woT": woTf,
            "hs_rows": np.ascontiguousarray(hs[P * r:P * (r + 1), :]),
            "b5": b5,
            "b3": b3,
        }
        in_maps.append(m)
    return in_maps


def run(inputs, trace=False):
    from concourse.bass_utils import run_bass_kernel_spmd

    nc, names = _get_compiled()
    logical = _prep_in_maps(inputs)
    in_maps = [{names[k]: v for k, v in m.items()} for m in logical]
    res = run_bass_kernel_spmd(nc, in_maps, list(range(NCORES)), trace=trace)
    outs = [res.results[r][names["out"]].astype(np.float32) for r in range(NCORES)]
    full = np.concatenate(outs, axis=0).reshape(1, N, HID)
    return full, res


def kernel(**inputs) -> np.ndarray:
    full, _ = run(inputs, trace=False)
    return full
